# revision 12
# baseline (speedup 1.0000x reference)
"""CLIPAttention (B=4, S=1024, D=768, H=12, causal) on 8 TRN2 NeuronCores.

Sharding: core c -> (batch b = c//2, head-group g = c%2).  Each core computes
6 heads of attention for one batch over the full sequence, then a PARTIAL
output projection (contraction over its 384 features).  The host sums the
two partial Z's of each batch pair — no on-device collectives.

Host-side algebraic folds (exact):
  - softmax scale folded into Wq, bq
  - K bias dropped (softmax is shift-invariant along k)
  - V bias folded through the output projection into bo_eff = bo + Wo @ bv
  - output bias added on only the g==0 core of each pair

Device layout: activations feature-major (transposed) everywhere, so no
on-device transposes.  Scores are computed as scores^T[k, q] per k-tile over
the causal q-suffix only (exact causal flop count); softmax skips the max
subtraction (scores are bounded), the causal mask is applied as a cheap
multiplicative binary mask on exp(scores) at the single diagonal 128x128
block per k-tile, and the softmax denominators ride along the PV matmul as a
65th "ones" column appended to V.  After PV, the reciprocal of the den row is
taken straight out of PSUM (DVE), partition-broadcast (GpSimd), and the
normalize multiply also reads the PV PSUM tile directly — no intermediate
SBUF copies.  The out-projection bias rides as a per-partition bias on the
ScalarE Identity-activation evacuation of the Z PSUM tiles.  The PE array is
pre-warmed with dummy matmuls during the initial DMA phase so the HAM clock
gate reaches 2.4 GHz before the first real matmul; input DMAs are split
chunk-wise across both HWDGE rings (sync + scalar) plus the GpSimd SWDGE
queue so the first projection starts ~1.5us in.  All matmul operands are
bf16 (PSUM accumulates f32); exp runs on ScalarE from f32 PSUM scores.
"""

import sys

import numpy as np

if "/opt/trn_rl_repo" not in sys.path:
    sys.path.insert(0, "/opt/trn_rl_repo")

B, S, D, H, HD = 4, 1024, 768, 12, 64
SCALE = HD**-0.5
NEG = -1e30
NS = [S - 128 * kt for kt in range(8)]  # q-suffix width per k-tile
N_WARM = 15  # dummy matmuls to pre-warm the PE HAM clock gate

_CACHE = {}


def _build_nc():
    import concourse.tile as tile
    from concourse import bacc, mybir

    f32 = mybir.dt.float32
    bf16 = mybir.dt.bfloat16
    PDT = bf16  # dtype of exp(P^T) and V (the PV matmul operands)
    Exp = mybir.ActivationFunctionType.Exp
    Ident = mybir.ActivationFunctionType.Identity
    MULT = mybir.AluOpType.mult

    nc = bacc.Bacc("TRN2", target_bir_lowering=False, debug=False, num_devices=8)
    y_d = nc.dram_tensor("y", [D, S], bf16, kind="ExternalInput")
    # wq/wk arrive pre-swizzled to SBUF layout [128p, m, kc, n] so each
    # per-m chunk is a single clean 2D DMA instruction
    wq_d = nc.dram_tensor("wq", [128, 3, 6, 128], bf16, kind="ExternalInput")
    wk_d = nc.dram_tensor("wk", [128, 3, 6, 128], bf16, kind="ExternalInput")
    wv_d = nc.dram_tensor("wv", [D, 384], bf16, kind="ExternalInput")
    wo_d = nc.dram_tensor("wo", [384, D], bf16, kind="ExternalInput")
    bq_d = nc.dram_tensor("bq", [384], f32, kind="ExternalInput")
    bo_d = nc.dram_tensor("bo", [D], f32, kind="ExternalInput")
    mask_d = nc.dram_tensor("mask", [128, 128], bf16, kind="ExternalInput")
    z_d = nc.dram_tensor("z", [D, S], f32, kind="ExternalOutput")

    with tile.TileContext(nc) as tc:
        from contextlib import ExitStack

        with ExitStack() as ctx:
            const = ctx.enter_context(tc.tile_pool(name="const", bufs=1))
            persist = ctx.enter_context(tc.tile_pool(name="persist", bufs=1))
            ptp = ctx.enter_context(tc.tile_pool(name="ptp", bufs=3))
            small = ctx.enter_context(tc.tile_pool(name="small", bufs=4))
            zpool = ctx.enter_context(tc.tile_pool(name="zpool", bufs=3))
            proj_ps = ctx.enter_context(
                tc.tile_pool(name="proj_ps", bufs=2, space="PSUM")
            )
            sc_ps = ctx.enter_context(tc.tile_pool(name="sc_ps", bufs=2, space="PSUM"))
            at_ps = ctx.enter_context(tc.tile_pool(name="at_ps", bufs=2, space="PSUM"))

            # ---------------- PE warmup (runs during the DMA phase) -------------
            warm_a = const.tile([128, 128], bf16, tag="warm_a", name="warm_a")
            nc.vector.memset(warm_a, 0.0)
            warm_ps = sc_ps.tile([128, 128], f32, tag="sc", name="warm_ps")
            for _ in range(N_WARM):
                nc.tensor.matmul(
                    warm_ps, lhsT=warm_a, rhs=warm_a, start=True, stop=True,
                    skip_group_check=True,
                )

            # ---------------- constant loads (split across DMA rings) ----------
            # sync ring: wq m0 chunk then even y tiles; scalar ring: wk m0
            # chunk then odd y tiles; gpsimd SWDGE: the rest.
            wq_t = const.tile([128, 3, 6, 128], bf16, tag="wq", name="wq")
            wk_t = const.tile([128, 3, 6, 128], bf16, tag="wk", name="wk")

            def wchunk(eng, t, d, m):
                eng.dma_start(out=t[:, m], in_=d.ap()[:, m])

            wchunk(nc.sync, wq_t, wq_d, 0)
            wchunk(nc.scalar, wk_t, wk_d, 0)
            y_sb = []
            for kc in range(6):
                t = const.tile([128, S], bf16, tag=f"y{kc}", name=f"y{kc}")
                eng = nc.sync if kc % 2 == 0 else nc.scalar
                eng.dma_start(out=t, in_=y_d.ap()[128 * kc : 128 * kc + 128, :])
                y_sb.append(t)
            wchunk(nc.gpsimd, wq_t, wq_d, 1)
            wchunk(nc.gpsimd, wk_t, wk_d, 1)
            wchunk(nc.gpsimd, wq_t, wq_d, 2)
            wchunk(nc.gpsimd, wk_t, wk_d, 2)
            wv_t = const.tile([128, 6, 384], bf16, tag="wv", name="wv")
            nc.gpsimd.dma_start(out=wv_t, in_=wv_d.ap().rearrange("(c p) m -> p c m", p=128))
            wo_t = const.tile([128, 3, D], bf16, tag="wo", name="wo")
            nc.gpsimd.dma_start(out=wo_t, in_=wo_d.ap().rearrange("(c p) m -> p c m", p=128))
            wq_sb = lambda kc, m: wq_t[:, m, kc, :]
            wk_sb = lambda kc, m: wk_t[:, m, kc, :]
            wv_sb = [wv_t[:, kc, :] for kc in range(6)]
            wo_sb = [wo_t[:, kc, :] for kc in range(3)]
            bq_t = const.tile([128, 3, 1], f32, tag="bq", name="bq")
            nc.gpsimd.dma_start(out=bq_t, in_=bq_d.ap().rearrange("(c p) -> p c", p=128)[:, :, None])
            bq_sb = [bq_t[:, m, :] for m in range(3)]
            bo_t = const.tile([128, 6, 1], f32, tag="bo", name="bo")
            nc.gpsimd.dma_start(out=bo_t, in_=bo_d.ap().rearrange("(c p) -> p c", p=128)[:, :, None])
            bo_sb = [bo_t[:, m, :] for m in range(6)]
            mask_sb = const.tile([128, 128], bf16, tag="mask", name="mask")
            nc.gpsimd.dma_start(out=mask_sb, in_=mask_d.ap())

            # preload the ACT exp table so the first real exp doesn't pay it
            warm = const.tile([1, 1], f32, tag="warm", name="warm")
            nc.vector.memset(warm, 0.0)
            nc.scalar.activation(out=warm, in_=warm, func=Exp)

            # ---------------- persistent activation tiles ----------------
            qT = [persist.tile([128, S], bf16, tag=f"q{m}", name=f"q{m}") for m in range(3)]
            kT = [persist.tile([128, S], bf16, tag=f"k{m}", name=f"k{m}") for m in range(3)]
            v_sb = [persist.tile([128, 390], PDT, tag=f"v{s}", name=f"v{s}") for s in range(8)]
            for s in range(8):
                vr = v_sb[s].rearrange("p (h c) -> p h c", c=65)
                nc.vector.memset(vr[:, :, 64:65], 1.0)
            attn_sb = [persist.tile([128, S], bf16, tag=f"at{m}", name=f"at{m}") for m in range(3)]

            # ---------------- QK projections (per m-tile) ----------------
            def qk_proj(m):
                for which in range(2):  # 0 = Q, 1 = K
                    w_sb = wq_sb if which == 0 else wk_sb
                    for n in range(2):
                        ps = proj_ps.tile([128, 512], f32, tag="proj", name="proj")
                        for kc in range(6):
                            nc.tensor.matmul(
                                ps,
                                lhsT=w_sb(kc, m),
                                rhs=y_sb[kc][:, 512 * n : 512 * n + 512],
                                start=(kc == 0),
                                stop=(kc == 5),
                            )
                        dst = (qT if which == 0 else kT)[m][
                            :, 512 * n : 512 * n + 512
                        ]
                        if which == 0:
                            nc.vector.tensor_scalar_add(
                                out=dst, in0=ps, scalar1=bq_sb[m]
                            )
                        else:
                            nc.vector.tensor_copy(out=dst, in_=ps)

            # ---------------- V projection ----------------
            def v_proj():
                for s in range(8):
                    ps = proj_ps.tile([128, 384], f32, tag="proj", name="proj")
                    for kc in range(6):
                        nc.tensor.matmul(
                            ps,
                            lhsT=y_sb[kc][:, 128 * s : 128 * s + 128],
                            rhs=wv_sb[kc],
                            start=(kc == 0),
                            stop=(kc == 5),
                        )
                    nc.vector.tensor_copy(
                        out=v_sb[s].rearrange("p (h c) -> p h c", c=65)[:, :, 0:64],
                        in_=ps.rearrange("p (h c) -> p h c", c=64),
                    )

            # ---------------- scores + exp for a head pair ----------------
            def scores_pair(p):
                pts = {}
                for kt in range(8):
                    N = NS[kt]
                    if kt < 4:
                        for hh in range(2):
                            po = 64 * hh
                            sct = sc_ps.tile([128, N], f32, tag="sc", name="sc")
                            c0 = 0
                            while c0 < N:
                                w = min(512, N - c0)
                                nc.tensor.matmul(
                                    sct[:, c0 : c0 + w],
                                    lhsT=kT[p][po : po + 64, 128 * kt : 128 * kt + 128],
                                    rhs=qT[p][
                                        po : po + 64, 128 * kt + c0 : 128 * kt + c0 + w
                                    ],
                                    start=True,
                                    stop=True,
                                    tile_position=(64 * hh, 0),
                                )
                                c0 += w
                            pt = ptp.tile([128, N], PDT, tag=f"pt{kt}h{hh}", name=f"pt{kt}h{hh}")
                            nc.scalar.activation(out=pt, in_=sct, func=Exp)
                            nc.vector.tensor_tensor(
                                out=pt[:, 0:128],
                                in0=pt[:, 0:128],
                                in1=mask_sb,
                                op=MULT,
                            )
                            pts[(kt, hh)] = (pt, 0)
                    else:
                        sct = sc_ps.tile([128, 512 + N], f32, tag="sc", name="sc")
                        for hh in range(2):
                            po = 64 * hh
                            o = 512 * hh
                            nc.tensor.matmul(
                                sct[:, o : o + N],
                                lhsT=kT[p][po : po + 64, 128 * kt : 128 * kt + 128],
                                rhs=qT[p][po : po + 64, 128 * kt :],
                                start=True,
                                stop=True,
                                tile_position=(64 * hh, 0),
                            )
                        pt = ptp.tile([128, 512 + N], PDT, tag=f"pt{kt}", name=f"pt{kt}")
                        nc.scalar.activation(out=pt, in_=sct, func=Exp)
                        for hh in range(2):
                            o = 512 * hh
                            nc.vector.tensor_tensor(
                                out=pt[:, o : o + 128],
                                in0=pt[:, o : o + 128],
                                in1=mask_sb,
                                op=MULT,
                            )
                        pts[(kt, 0)] = (pt, 0)
                        pts[(kt, 1)] = (pt, 512)
                return pts

            # ---------------- PV + normalize for a head pair ----------------
            def pv_one(p, hh, Bb, pts):
                h = 2 * p + hh
                po = 64 * hh
                if Bb == 0:
                    rot = (2 * p + hh) % 3
                else:
                    rot = (2 * p + hh) % 2
                if rot == 0:
                    at = at_ps.tile([65, 512], f32, tag="at", name="at")
                elif rot == 1:
                    at = sc_ps.tile([65, 512], f32, tag="sc", name="at2")
                else:
                    at = proj_ps.tile([65, 512], f32, tag="proj", name="at3")
                Jmax = 4 * Bb + 3
                for kt in range(Jmax + 1):
                    J0 = max(kt, 4 * Bb)  # first region this ktile touches
                    nJ = Jmax - J0 + 1
                    pt, base = pts[(kt, hh)]
                    co = base + 128 * (J0 - kt)
                    nc.tensor.matmul(
                        at[0:65, 128 * (J0 - 4 * Bb) : 128 * (J0 - 4 * Bb) + 128 * nJ],
                        lhsT=v_sb[kt][:, 65 * h : 65 * h + 65],
                        rhs=pt[:, co : co + 128 * nJ],
                        start=(kt == 0),
                        stop=(kt == Jmax),
                        skip_group_check=True,
                    )
                # normalize: den row PSUM->SBUF on ScalarE (the only copy),
                # reciprocal + broadcast, then the multiply reads the PV
                # PSUM tile directly (no au evacuation copy).
                # reciprocal_approx_fast can NOT read PSUM (garbage results).
                den = small.tile([1, 512], f32, tag="den", name="den")
                nc.scalar.copy(out=den, in_=at[64:65, 0:512])
                r = small.tile([1, 512], f32, tag="r", name="r")
                nc.vector.reciprocal_approx_fast(out=r, in_=den)
                rb = small.tile([64, 512], f32, tag="rb", name="rb")
                nc.gpsimd.partition_broadcast(rb, r)
                nc.vector.tensor_tensor(
                    out=attn_sb[p][po : po + 64, 512 * Bb : 512 * Bb + 512],
                    in0=at[0:64, 0:512],
                    in1=rb,
                    op=MULT,
                )

            # ---------------- out projection (partial) ----------------
            def out_proj(n):
                if n == 0:
                    # between PV waves: tight per-m bursts on the proj slots
                    for m in range(6):
                        ps = proj_ps.tile([128, 512], f32, tag="proj", name="proj")
                        for kc in range(3):
                            nc.tensor.matmul(
                                ps,
                                lhsT=wo_sb[kc][:, 128 * m : 128 * m + 128],
                                rhs=attn_sb[kc][:, 512 * n : 512 * n + 512],
                                start=(kc == 0),
                                stop=(kc == 2),
                            )
                        zt = zpool.tile([128, 512], f32, tag="z", name="z")
                        nc.scalar.activation(out=zt, in_=ps, func=Ident, bias=bo_sb[m])
                        eng = nc.sync if m % 2 == 0 else nc.scalar
                        eng.dma_start(
                            out=z_d.ap()[
                                128 * m : 128 * m + 128, 512 * n : 512 * n + 512
                            ],
                            in_=zt,
                        )
                else:
                    # tail pass: all PV psum slots are free — six kc-split
                    # chains in flight; each kc wave gates only on that
                    # pair's norms, so just the last wave waits the last norm
                    pss = []
                    for m in range(6):
                        if m < 2:
                            ps = proj_ps.tile([128, 512], f32, tag="proj", name="zp")
                        elif m < 4:
                            ps = at_ps.tile([128, 512], f32, tag="at", name="za")
                        else:
                            ps = sc_ps.tile([128, 512], f32, tag="sc", name="zs")
                        pss.append(ps)
                    for kc in range(3):
                        for m in range(6):
                            nc.tensor.matmul(
                                pss[m],
                                lhsT=wo_sb[kc][:, 128 * m : 128 * m + 128],
                                rhs=attn_sb[kc][:, 512 * n : 512 * n + 512],
                                start=(kc == 0),
                                stop=(kc == 2),
                                skip_group_check=True,
                            )
                    for m in range(6):
                        zt = zpool.tile([128, 512], f32, tag="z", name="z")
                        nc.scalar.activation(
                            out=zt, in_=pss[m], func=Ident, bias=bo_sb[m]
                        )
                        eng = nc.sync if m % 2 == 0 else nc.scalar
                        eng.dma_start(
                            out=z_d.ap()[
                                128 * m : 128 * m + 128, 512 * n : 512 * n + 512
                            ],
                            in_=zt,
                        )

            # ---------------- emission order ----------------
            # v_proj is pulled before qk_proj(2) and the PV waves are spread
            # into the scores phase, so every normalize chain overlaps dense
            # tensor work instead of stacking up at the tail.
            all_pts = {}
            qk_proj(0)
            all_pts[0] = scores_pair(0)
            qk_proj(1)
            all_pts[1] = scores_pair(1)
            v_proj()
            for hh in range(2):
                pv_one(0, hh, 0, all_pts[0])
            qk_proj(2)
            for hh in range(2):
                pv_one(1, hh, 0, all_pts[1])
            all_pts[2] = scores_pair(2)
            for hh in range(2):
                pv_one(2, hh, 0, all_pts[2])
            for hh in range(2):
                pv_one(0, hh, 1, all_pts[0])
            out_proj(0)
            for hh in range(2):
                pv_one(1, hh, 1, all_pts[1])
            for hh in range(2):
                pv_one(2, hh, 1, all_pts[2])
            out_proj(1)

    nc.compile()
    return nc


def _get_nc():
    if "nc" not in _CACHE:
        _CACHE["nc"] = _build_nc()
    return _CACHE["nc"]


def _host_prep(inputs):
    import ml_dtypes

    bf = ml_dtypes.bfloat16
    hs = np.ascontiguousarray(np.asarray(inputs["hidden_states"], np.float32))
    Wq = np.asarray(inputs["Wq"], np.float32)
    bq = np.asarray(inputs["bq"], np.float32)
    Wk = np.asarray(inputs["Wk"], np.float32)
    Wv = np.asarray(inputs["Wv"], np.float32)
    bv = np.asarray(inputs["bv"], np.float32)
    Wo = np.asarray(inputs["Wo"], np.float32)
    bo = np.asarray(inputs["bo"], np.float32)

    bo_eff = (bo + Wo @ bv).astype(np.float32)
    zeros_bo = np.zeros_like(bo_eff)
    mask = (np.arange(128)[:, None] <= np.arange(128)[None, :]).astype(bf)

    wq_g, wk_g, wv_g, wo_g, bq_g = [], [], [], [], []
    for g in range(2):
        r0 = 384 * g
        wqT = np.ascontiguousarray((Wq[r0 : r0 + 384, :] * SCALE).T.astype(bf))
        wkT = np.ascontiguousarray(Wk[r0 : r0 + 384, :].T.astype(bf))
        # pre-swizzle to SBUF layout [128p, m, kc, n]
        wq_g.append(
            np.ascontiguousarray(wqT.reshape(6, 128, 3, 128).transpose(1, 2, 0, 3))
        )
        wk_g.append(
            np.ascontiguousarray(wkT.reshape(6, 128, 3, 128).transpose(1, 2, 0, 3))
        )
        wv_g.append(np.ascontiguousarray(Wv[r0 : r0 + 384, :].T.astype(bf)))
        wo_g.append(np.ascontiguousarray(Wo[:, r0 : r0 + 384].T.astype(bf)))
        bq_g.append(np.ascontiguousarray(bq[r0 : r0 + 384] * SCALE))

    yb = [np.ascontiguousarray(hs[b].T.astype(bf)) for b in range(B)]

    in_maps = []
    for c in range(8):
        b, g = c // 2, c % 2
        in_maps.append(
            {
                "y": yb[b],
                "wq": wq_g[g],
                "wk": wk_g[g],
                "wv": wv_g[g],
                "wo": wo_g[g],
                "bq": bq_g[g],
                "bo": bo_eff if g == 0 else zeros_bo,
                "mask": mask,
            }
        )
    return in_maps


def kernel(**inputs):
    from concourse.bass_utils import run_bass_kernel_spmd

    nc = _get_nc()
    in_maps = _host_prep(inputs)
    res = run_bass_kernel_spmd(nc, in_maps, core_ids=list(range(8)))
    zs = [res.results[i]["z"] for i in range(8)]
    out = np.stack(
        [(zs[2 * b].astype(np.float32) + zs[2 * b + 1].astype(np.float32)).T
         for b in range(B)]
    )
    return np.ascontiguousarray(out.astype(np.float32))


# revision 15
# speedup vs baseline: 1.1022x; 1.1022x over previous
"""CLIPAttention (B=4, S=1024, D=768, H=12, causal) on 8 TRN2 NeuronCores.

Sharding: core c -> (batch b = c//2, head-group g = c%2).  Each core computes
6 heads of attention for one batch over the full sequence, then a PARTIAL
output projection (contraction over its 384 features).  The host sums the
two partial Z's of each batch pair — no on-device collectives.

Host-side algebraic folds (exact):
  - softmax scale folded into Wq, bq
  - K bias dropped (softmax is shift-invariant along k)
  - V bias folded through the output projection into bo_eff = bo + Wo @ bv
  - output bias added on only the g==0 core of each pair

Device layout: activations feature-major (transposed) everywhere, so no
on-device transposes.  Scores are computed as scores^T[k, q] per k-tile over
the causal q-suffix only (exact causal flop count); softmax skips the max
subtraction (scores are bounded), the causal mask is applied as a cheap
multiplicative binary mask on exp(scores) at the single diagonal 128x128
block per k-tile, and the softmax denominators ride along the PV matmul as a
65th "ones" column appended to V.  After PV, the reciprocal of the den row is
taken straight out of PSUM (DVE), partition-broadcast (GpSimd), and the
normalize multiply also reads the PV PSUM tile directly — no intermediate
SBUF copies.  The out-projection bias rides as a per-partition bias on the
ScalarE Identity-activation evacuation of the Z PSUM tiles.  The PE array is
pre-warmed with dummy matmuls during the initial DMA phase so the HAM clock
gate reaches 2.4 GHz before the first real matmul; input DMAs are split
chunk-wise across both HWDGE rings (sync + scalar) plus the GpSimd SWDGE
queue so the first projection starts ~1.5us in.  All matmul operands are
bf16 (PSUM accumulates f32); exp runs on ScalarE from f32 PSUM scores.
"""

import sys

import numpy as np

if "/opt/trn_rl_repo" not in sys.path:
    sys.path.insert(0, "/opt/trn_rl_repo")

B, S, D, H, HD = 4, 1024, 768, 12, 64
SCALE = HD**-0.5
NEG = -1e30
NS = [S - 128 * kt for kt in range(8)]  # q-suffix width per k-tile
N_WARM = 15  # dummy matmuls to pre-warm the PE HAM clock gate

_CACHE = {}


def _build_nc():
    import concourse.tile as tile
    from concourse import bacc, mybir

    f32 = mybir.dt.float32
    bf16 = mybir.dt.bfloat16
    PDT = bf16  # dtype of exp(P^T) and V (the PV matmul operands)
    Exp = mybir.ActivationFunctionType.Exp
    Ident = mybir.ActivationFunctionType.Identity
    MULT = mybir.AluOpType.mult

    nc = bacc.Bacc("TRN2", target_bir_lowering=False, debug=False, num_devices=8)
    y_d = nc.dram_tensor("y", [D, S], bf16, kind="ExternalInput")
    # wq/wk arrive pre-swizzled to SBUF layout [128p, m, kc, n] so each
    # per-m chunk is a single clean 2D DMA instruction
    wq_d = nc.dram_tensor("wq", [128, 3, 6, 128], bf16, kind="ExternalInput")
    wk_d = nc.dram_tensor("wk", [128, 3, 6, 128], bf16, kind="ExternalInput")
    wv_d = nc.dram_tensor("wv", [D, 384], bf16, kind="ExternalInput")
    wo_d = nc.dram_tensor("wo", [384, D], bf16, kind="ExternalInput")
    bq_d = nc.dram_tensor("bq", [384], f32, kind="ExternalInput")
    bo_d = nc.dram_tensor("bo", [D], f32, kind="ExternalInput")
    mask_d = nc.dram_tensor("mask", [128, 128], bf16, kind="ExternalInput")
    z_d = nc.dram_tensor("z", [D, S], f32, kind="ExternalOutput")

    with tile.TileContext(nc) as tc:
        from contextlib import ExitStack

        with ExitStack() as ctx:
            const = ctx.enter_context(tc.tile_pool(name="const", bufs=1))
            persist = ctx.enter_context(tc.tile_pool(name="persist", bufs=1))
            ptp = ctx.enter_context(tc.tile_pool(name="ptp", bufs=3))
            small = ctx.enter_context(tc.tile_pool(name="small", bufs=4))
            zpool = ctx.enter_context(tc.tile_pool(name="zpool", bufs=3))
            proj_ps = ctx.enter_context(
                tc.tile_pool(name="proj_ps", bufs=2, space="PSUM")
            )
            sc_ps = ctx.enter_context(tc.tile_pool(name="sc_ps", bufs=2, space="PSUM"))
            at_ps = ctx.enter_context(tc.tile_pool(name="at_ps", bufs=2, space="PSUM"))

            # ---------------- PE warmup (runs during the DMA phase) -------------
            warm_a = const.tile([128, 128], bf16, tag="warm_a", name="warm_a")
            nc.vector.memset(warm_a, 0.0)
            warm_ps = sc_ps.tile([128, 128], f32, tag="sc", name="warm_ps")
            for _ in range(N_WARM):
                nc.tensor.matmul(
                    warm_ps, lhsT=warm_a, rhs=warm_a, start=True, stop=True,
                    skip_group_check=True,
                )

            # ---------------- constant loads (two HWDGE rings, priority order) --
            # Criticals (wq/wk m0, y) first so the QK phase starts ~7.5us;
            # wv/wo/smalls trail on the same rings.  GpSimd issues NO DMAs —
            # its queue stays clear for the partition_broadcasts.
            wq_t = const.tile([128, 3, 6, 128], bf16, tag="wq", name="wq")
            wk_t = const.tile([128, 3, 6, 128], bf16, tag="wk", name="wk")
            bq_t = const.tile([128, 3, 1], f32, tag="bq", name="bq")
            wv_t = const.tile([128, 6, 384], bf16, tag="wv", name="wv")
            wo_t = const.tile([128, 3, D], bf16, tag="wo", name="wo")
            bo_t = const.tile([128, 6, 1], f32, tag="bo", name="bo")
            mask_sb = const.tile([128, 128], bf16, tag="mask", name="mask")

            def wchunk(eng, t, d, m):
                eng.dma_start(out=t[:, m], in_=d.ap()[:, m])

            y_sb = [
                const.tile([128, S], bf16, tag=f"y{kc}", name=f"y{kc}")
                for kc in range(6)
            ]
            # sync ring
            wchunk(nc.sync, wq_t, wq_d, 0)
            for kc in (0, 2, 4):
                nc.sync.dma_start(out=y_sb[kc], in_=y_d.ap()[128 * kc : 128 * kc + 128, :])
            wchunk(nc.sync, wq_t, wq_d, 1)
            wchunk(nc.sync, wq_t, wq_d, 2)
            nc.sync.dma_start(out=wv_t, in_=wv_d.ap().rearrange("(c p) m -> p c m", p=128))
            nc.sync.dma_start(out=bo_t, in_=bo_d.ap().rearrange("(c p) -> p c", p=128)[:, :, None])
            nc.sync.dma_start(out=mask_sb, in_=mask_d.ap())
            # scalar ring
            nc.scalar.dma_start(out=bq_t, in_=bq_d.ap().rearrange("(c p) -> p c", p=128)[:, :, None])
            wchunk(nc.scalar, wk_t, wk_d, 0)
            for kc in (1, 3, 5):
                nc.scalar.dma_start(out=y_sb[kc], in_=y_d.ap()[128 * kc : 128 * kc + 128, :])
            wchunk(nc.scalar, wk_t, wk_d, 1)
            wchunk(nc.scalar, wk_t, wk_d, 2)
            nc.scalar.dma_start(out=wo_t, in_=wo_d.ap().rearrange("(c p) m -> p c m", p=128))

            wq_sb = lambda kc, m: wq_t[:, m, kc, :]
            wk_sb = lambda kc, m: wk_t[:, m, kc, :]
            wv_sb = [wv_t[:, kc, :] for kc in range(6)]
            wo_sb = [wo_t[:, kc, :] for kc in range(3)]
            bq_sb = [bq_t[:, m, :] for m in range(3)]
            bo_sb = [bo_t[:, m, :] for m in range(6)]

            # preload the ACT exp table so the first real exp doesn't pay it
            warm = const.tile([1, 1], f32, tag="warm", name="warm")
            nc.vector.memset(warm, 0.0)
            nc.scalar.activation(out=warm, in_=warm, func=Exp)

            # ---------------- persistent activation tiles ----------------
            qT = [persist.tile([128, S], bf16, tag=f"q{m}", name=f"q{m}") for m in range(3)]
            kT = [persist.tile([128, S], bf16, tag=f"k{m}", name=f"k{m}") for m in range(3)]
            v_sb = [persist.tile([128, 390], PDT, tag=f"v{s}", name=f"v{s}") for s in range(8)]
            for s in range(8):
                vr = v_sb[s].rearrange("p (h c) -> p h c", c=65)
                nc.vector.memset(vr[:, :, 64:65], 1.0)
            attn_sb = [persist.tile([128, S], bf16, tag=f"at{m}", name=f"at{m}") for m in range(3)]

            # ---------------- QK projections (per m-tile) ----------------
            def qk_proj(m):
                for which in range(2):  # 0 = Q, 1 = K
                    w_sb = wq_sb if which == 0 else wk_sb
                    for n in range(2):
                        ps = proj_ps.tile([128, 512], f32, tag="proj", name="proj")
                        for kc in range(6):
                            nc.tensor.matmul(
                                ps,
                                lhsT=w_sb(kc, m),
                                rhs=y_sb[kc][:, 512 * n : 512 * n + 512],
                                start=(kc == 0),
                                stop=(kc == 5),
                            )
                        dst = (qT if which == 0 else kT)[m][
                            :, 512 * n : 512 * n + 512
                        ]
                        if which == 0:
                            nc.vector.tensor_scalar_add(
                                out=dst, in0=ps, scalar1=bq_sb[m]
                            )
                        else:
                            nc.vector.tensor_copy(out=dst, in_=ps)

            # ---------------- V projection ----------------
            def v_proj():
                for s in range(8):
                    ps = proj_ps.tile([128, 384], f32, tag="proj", name="proj")
                    for kc in range(6):
                        nc.tensor.matmul(
                            ps,
                            lhsT=y_sb[kc][:, 128 * s : 128 * s + 128],
                            rhs=wv_sb[kc],
                            start=(kc == 0),
                            stop=(kc == 5),
                        )
                    nc.vector.tensor_copy(
                        out=v_sb[s].rearrange("p (h c) -> p h c", c=65)[:, :, 0:64],
                        in_=ps.rearrange("p (h c) -> p h c", c=64),
                    )

            # ---------------- scores + exp for a head pair ----------------
            def scores_pair(p):
                pts = {}
                for kt in range(8):
                    N = NS[kt]
                    if kt < 4:
                        for hh in range(2):
                            po = 64 * hh
                            sct = sc_ps.tile([128, N], f32, tag="sc", name="sc")
                            c0 = 0
                            while c0 < N:
                                w = min(512, N - c0)
                                nc.tensor.matmul(
                                    sct[:, c0 : c0 + w],
                                    lhsT=kT[p][po : po + 64, 128 * kt : 128 * kt + 128],
                                    rhs=qT[p][
                                        po : po + 64, 128 * kt + c0 : 128 * kt + c0 + w
                                    ],
                                    start=True,
                                    stop=True,
                                    tile_position=(64 * hh, 0),
                                )
                                c0 += w
                            pt = ptp.tile([128, N], PDT, tag=f"pt{kt}h{hh}", name=f"pt{kt}h{hh}")
                            nc.scalar.activation(out=pt, in_=sct, func=Exp)
                            nc.vector.tensor_tensor(
                                out=pt[:, 0:128],
                                in0=pt[:, 0:128],
                                in1=mask_sb,
                                op=MULT,
                            )
                            pts[(kt, hh)] = (pt, 0)
                    else:
                        sct = sc_ps.tile([128, 512 + N], f32, tag="sc", name="sc")
                        for hh in range(2):
                            po = 64 * hh
                            o = 512 * hh
                            nc.tensor.matmul(
                                sct[:, o : o + N],
                                lhsT=kT[p][po : po + 64, 128 * kt : 128 * kt + 128],
                                rhs=qT[p][po : po + 64, 128 * kt :],
                                start=True,
                                stop=True,
                                tile_position=(64 * hh, 0),
                            )
                        pt = ptp.tile([128, 512 + N], PDT, tag=f"pt{kt}", name=f"pt{kt}")
                        nc.scalar.activation(out=pt, in_=sct, func=Exp)
                        for hh in range(2):
                            o = 512 * hh
                            nc.vector.tensor_tensor(
                                out=pt[:, o : o + 128],
                                in0=pt[:, o : o + 128],
                                in1=mask_sb,
                                op=MULT,
                            )
                        pts[(kt, 0)] = (pt, 0)
                        pts[(kt, 1)] = (pt, 512)
                return pts

            # ---------------- PV + normalize for a head pair ----------------
            def pv_one(p, hh, Bb, pts):
                h = 2 * p + hh
                po = 64 * hh
                if Bb == 0:
                    rot = (2 * p + hh) % 3
                else:
                    rot = (2 * p + hh) % 2
                if rot == 0:
                    at = at_ps.tile([65, 512], f32, tag="at", name="at")
                elif rot == 1:
                    at = sc_ps.tile([65, 512], f32, tag="sc", name="at2")
                else:
                    at = proj_ps.tile([65, 512], f32, tag="proj", name="at3")
                Jmax = 4 * Bb + 3
                for kt in range(Jmax + 1):
                    J0 = max(kt, 4 * Bb)  # first region this ktile touches
                    nJ = Jmax - J0 + 1
                    pt, base = pts[(kt, hh)]
                    co = base + 128 * (J0 - kt)
                    nc.tensor.matmul(
                        at[0:65, 128 * (J0 - 4 * Bb) : 128 * (J0 - 4 * Bb) + 128 * nJ],
                        lhsT=v_sb[kt][:, 65 * h : 65 * h + 65],
                        rhs=pt[:, co : co + 128 * nJ],
                        start=(kt == 0),
                        stop=(kt == Jmax),
                        skip_group_check=True,
                    )
                # normalize: den row PSUM->SBUF on ScalarE (the only copy),
                # reciprocal + broadcast, then the multiply reads the PV
                # PSUM tile directly (no au evacuation copy).
                # reciprocal_approx_fast can NOT read PSUM (garbage results).
                den = small.tile([1, 512], f32, tag="den", name="den")
                nc.scalar.copy(out=den, in_=at[64:65, 0:512])
                r = small.tile([1, 512], f32, tag="r", name="r")
                nc.vector.reciprocal_approx_fast(out=r, in_=den)
                rb = small.tile([64, 512], f32, tag="rb", name="rb")
                nc.gpsimd.partition_broadcast(rb, r)
                nc.vector.tensor_tensor(
                    out=attn_sb[p][po : po + 64, 512 * Bb : 512 * Bb + 512],
                    in0=at[0:64, 0:512],
                    in1=rb,
                    op=MULT,
                )

            # ---------------- out projection (partial) ----------------
            def out_proj(n):
                if n == 0:
                    # between PV waves: tight per-m bursts on the proj slots
                    for m in range(6):
                        ps = proj_ps.tile([128, 512], f32, tag="proj", name="proj")
                        for kc in range(3):
                            nc.tensor.matmul(
                                ps,
                                lhsT=wo_sb[kc][:, 128 * m : 128 * m + 128],
                                rhs=attn_sb[kc][:, 512 * n : 512 * n + 512],
                                start=(kc == 0),
                                stop=(kc == 2),
                            )
                        zt = zpool.tile([128, 512], f32, tag="z", name="z")
                        nc.scalar.activation(out=zt, in_=ps, func=Ident, bias=bo_sb[m])
                        eng = nc.sync if m % 2 == 0 else nc.scalar
                        eng.dma_start(
                            out=z_d.ap()[
                                128 * m : 128 * m + 128, 512 * n : 512 * n + 512
                            ],
                            in_=zt,
                        )
                else:
                    # tail pass: all PV psum slots are free — six kc-split
                    # chains in flight; each kc wave gates only on that
                    # pair's norms, so just the last wave waits the last norm
                    pss = []
                    for m in range(6):
                        if m < 2:
                            ps = proj_ps.tile([128, 512], f32, tag="proj", name="zp")
                        elif m < 4:
                            ps = at_ps.tile([128, 512], f32, tag="at", name="za")
                        else:
                            ps = sc_ps.tile([128, 512], f32, tag="sc", name="zs")
                        pss.append(ps)
                    for kc in range(3):
                        for m in range(6):
                            nc.tensor.matmul(
                                pss[m],
                                lhsT=wo_sb[kc][:, 128 * m : 128 * m + 128],
                                rhs=attn_sb[kc][:, 512 * n : 512 * n + 512],
                                start=(kc == 0),
                                stop=(kc == 2),
                                skip_group_check=True,
                            )
                    for m in range(6):
                        zt = zpool.tile([128, 512], f32, tag="z", name="z")
                        # alternate evac engines so the 6 tail evacuations
                        # don't serialize on ScalarE
                        if m % 2 == 0:
                            nc.scalar.activation(
                                out=zt, in_=pss[m], func=Ident, bias=bo_sb[m]
                            )
                        else:
                            nc.vector.tensor_scalar_add(
                                out=zt, in0=pss[m], scalar1=bo_sb[m]
                            )
                        eng = nc.sync if m % 2 == 0 else nc.scalar
                        eng.dma_start(
                            out=z_d.ap()[
                                128 * m : 128 * m + 128, 512 * n : 512 * n + 512
                            ],
                            in_=zt,
                        )

            # ---------------- emission order ----------------
            # All scores (and so ALL exps) are emitted before any PV wave:
            # the scalar FIFO becomes [exps][den copies][z evacs] with no
            # interleaving, so a stalled normalize chain can never convoy
            # the exp pipeline.  Pair-2's exp tail overlaps v_proj + the
            # first wave-0 PVs on the tensor engine.
            all_pts = {}
            qk_proj(0)
            all_pts[0] = scores_pair(0)
            qk_proj(1)
            all_pts[1] = scores_pair(1)
            qk_proj(2)
            all_pts[2] = scores_pair(2)
            v_proj()
            for p in range(3):
                for hh in range(2):
                    pv_one(p, hh, 0, all_pts[p])
            for hh in range(2):
                pv_one(0, hh, 1, all_pts[0])
            out_proj(0)
            for p in range(1, 3):
                for hh in range(2):
                    pv_one(p, hh, 1, all_pts[p])
            out_proj(1)

    nc.compile()
    return nc


def _get_nc():
    if "nc" not in _CACHE:
        _CACHE["nc"] = _build_nc()
    return _CACHE["nc"]


def _host_prep(inputs):
    import ml_dtypes

    bf = ml_dtypes.bfloat16
    hs = np.ascontiguousarray(np.asarray(inputs["hidden_states"], np.float32))
    Wq = np.asarray(inputs["Wq"], np.float32)
    bq = np.asarray(inputs["bq"], np.float32)
    Wk = np.asarray(inputs["Wk"], np.float32)
    Wv = np.asarray(inputs["Wv"], np.float32)
    bv = np.asarray(inputs["bv"], np.float32)
    Wo = np.asarray(inputs["Wo"], np.float32)
    bo = np.asarray(inputs["bo"], np.float32)

    bo_eff = (bo + Wo @ bv).astype(np.float32)
    zeros_bo = np.zeros_like(bo_eff)
    mask = (np.arange(128)[:, None] <= np.arange(128)[None, :]).astype(bf)

    wq_g, wk_g, wv_g, wo_g, bq_g = [], [], [], [], []
    for g in range(2):
        r0 = 384 * g
        wqT = np.ascontiguousarray((Wq[r0 : r0 + 384, :] * SCALE).T.astype(bf))
        wkT = np.ascontiguousarray(Wk[r0 : r0 + 384, :].T.astype(bf))
        # pre-swizzle to SBUF layout [128p, m, kc, n]
        wq_g.append(
            np.ascontiguousarray(wqT.reshape(6, 128, 3, 128).transpose(1, 2, 0, 3))
        )
        wk_g.append(
            np.ascontiguousarray(wkT.reshape(6, 128, 3, 128).transpose(1, 2, 0, 3))
        )
        wv_g.append(np.ascontiguousarray(Wv[r0 : r0 + 384, :].T.astype(bf)))
        wo_g.append(np.ascontiguousarray(Wo[:, r0 : r0 + 384].T.astype(bf)))
        bq_g.append(np.ascontiguousarray(bq[r0 : r0 + 384] * SCALE))

    yb = [np.ascontiguousarray(hs[b].T.astype(bf)) for b in range(B)]

    in_maps = []
    for c in range(8):
        b, g = c // 2, c % 2
        in_maps.append(
            {
                "y": yb[b],
                "wq": wq_g[g],
                "wk": wk_g[g],
                "wv": wv_g[g],
                "wo": wo_g[g],
                "bq": bq_g[g],
                "bo": bo_eff if g == 0 else zeros_bo,
                "mask": mask,
            }
        )
    return in_maps


def kernel(**inputs):
    from concourse.bass_utils import run_bass_kernel_spmd

    nc = _get_nc()
    in_maps = _host_prep(inputs)
    res = run_bass_kernel_spmd(nc, in_maps, core_ids=list(range(8)))
    zs = [res.results[i]["z"] for i in range(8)]
    out = np.stack(
        [(zs[2 * b].astype(np.float32) + zs[2 * b + 1].astype(np.float32)).T
         for b in range(B)]
    )
    return np.ascontiguousarray(out.astype(np.float32))
